# revision 7
# baseline (speedup 1.0000x reference)
"""Multi-head attention (B=2, S=2048, D=1024, H=16) on 8 TRN2 NeuronCores.

Sharding: core c handles batch b = c//4 and head-group g = c%4 (4 heads each).
Each core computes its heads' attention and a partial output projection
(row-parallel W_o); the host sums the 4 partials per batch and adds b_o.

Device-side layout trick: everything runs in the "transposed world".
The host passes x[b].T and mask.T, so the QK projection produces Q^T/K^T
directly (no on-device transposes), scores are computed as S^T = K·Q^T with
keys on partitions, softmax denominators come from a ones-column appended to
V, and the output projection produces out^T which the host transposes back.
"""

import numpy as np

import concourse.bass as bass  # noqa: F401
import concourse.mybir as mybir
import concourse.tile as tile
from concourse import bacc
from concourse.bass import ds, ts
from concourse.bass_utils import run_bass_kernel_spmd

B, S, D, H = 2, 2048, 1024, 16
HD = D // H  # 64
HPC = 4      # heads per core
NCORES = 8
F32R = mybir.dt.float32r
AF = mybir.ActivationFunctionType

_CACHE = {}


def _build():
    nc = bacc.Bacc(None, target_bir_lowering=False, debug=False)
    xT = nc.dram_tensor("xT", [D, S], F32R, kind="ExternalInput")
    wqk = nc.dram_tensor("wqk", [D, 512], F32R, kind="ExternalInput")
    bqk = nc.dram_tensor("bqk", [128, 4], F32R, kind="ExternalInput")
    wv = nc.dram_tensor("wv", [D, 256], F32R, kind="ExternalInput")
    bv = nc.dram_tensor("bv", [1, 256], F32R, kind="ExternalInput")
    wo = nc.dram_tensor("wo", [256, D], F32R, kind="ExternalInput")
    maskT = nc.dram_tensor("maskT", [S, S], F32R, kind="ExternalInput")
    outT = nc.dram_tensor("outT", [D, S], mybir.dt.float32, kind="ExternalOutput")

    with tile.TileContext(nc) as tc:
        with (
            tc.tile_pool(name="big", bufs=1) as bigp,
            tc.tile_pool(name="wqkp", bufs=1) as wqkp,
            tc.tile_pool(name="wvp", bufs=1) as wvp,
            tc.tile_pool(name="wop", bufs=1) as wop,
            tc.tile_pool(name="qkp", bufs=1) as qkp,
            tc.tile_pool(name="vp", bufs=1) as vp,
            tc.tile_pool(name="valsp", bufs=1) as valsp,
            tc.tile_pool(name="attnp", bufs=3) as attnp,
            tc.tile_pool(name="smallp", bufs=2) as smallp,
            tc.tile_pool(name="constp", bufs=1) as constp,
        ):
            ones_t = constp.tile([1, 128], F32R)
            nc.gpsimd.memset(ones_t[:].bitcast(mybir.dt.float32), 1.0)
            b_sb = constp.tile([128, 4], F32R)
            nc.sync.dma_start(b_sb[:], bqk[:])
            bv_sb = constp.tile([1, 256], F32R)
            nc.sync.dma_start(bv_sb[:], bv[:])

            wqk_sb = wqkp.tile([128, 8, 512], F32R)
            nc.sync.dma_start(wqk_sb[:], wqk[:].rearrange("(dc p) c -> p dc c", p=128))
            wv_sb = wvp.tile([128, 8, 256], F32R)
            nc.sync.dma_start(wv_sb[:], wv[:].rearrange("(dc p) c -> p dc c", p=128))
            wo_sb = wop.tile([128, 2, D], F32R)
            nc.sync.dma_start(wo_sb[:], wo[:].rearrange("(kc p) d -> p kc d", p=128))

            xt_sb = bigp.tile([128, 8, S], F32R, tag="big")
            nc.sync.dma_start(xt_sb[:], xT[:].rearrange("(dc p) s -> p dc s", p=128))

            # qk_sb rows (partition+chunk) = projected qkv column:
            # chunk 0: q of heads 0,1; chunk 1: q of heads 2,3;
            # chunk 2: k of heads 0,1; chunk 3: k of heads 2,3.
            qk_sb = qkp.tile([128, 4, S], F32R)
            # v_sb[s%128, s//128, h, 0:64] = V; [..., 64] = 1.0 (denominator col)
            v_sb = vp.tile([128, 16, HPC, 65], F32R)
            vals_sb = valsp.tile([128, 2, S], F32R)

            nc.gpsimd.memset(v_sb[:, :, :, 64:65].bitcast(mybir.dt.float32), 1.0)

            # ---- V projection (natural layout): V[s, c] = x @ Wv + bv ----
            with tc.tile_pool(name="psv", bufs=2, space="PSUM") as psv:
                for sb_i in range(16):
                    ps = psv.tile([128, 256], mybir.dt.float32, tag="psv")
                    for dc in range(8):
                        nc.tensor.matmul(
                            ps[:], xt_sb[:, dc, ts(sb_i, 128)], wv_sb[:, dc, :],
                            start=(dc == 0), stop=False,
                        )
                    # += ones[s] * bv  (rank-1 bias add)
                    nc.tensor.matmul(ps[:], ones_t[:], bv_sb[:], start=False, stop=True)
                    nc.scalar.activation(
                        v_sb[:, sb_i, :, 0:64],
                        ps[:].rearrange("p (h e) -> p h e", h=HPC),
                        AF.Copy,
                    )

            # ---- QK projection (transposed): qkT[c, s] = W^T x^T (+bias) ----
            with tc.tile_pool(name="psqk", bufs=2, space="PSUM") as psqk:
                for cc in range(4):
                    ps = psqk.tile([128, S], mybir.dt.float32, tag="psqk")
                    for dc in range(8):
                        lhsT = wqk_sb[:, dc, ts(cc, 128)]
                        for nn in range(4):
                            nc.tensor.matmul(
                                ps[:, ts(nn, 512)], lhsT, xt_sb[:, dc, ts(nn, 512)],
                                start=(dc == 0), stop=(dc == 7),
                            )
                    nc.scalar.activation(
                        qk_sb[:, cc, :], ps[:], AF.Identity, bias=b_sb[:, cc : cc + 1]
                    )

            # ---- attention, in [k, q] layout, q processed in two halves ----
            with (
                tc.tile_pool(name="pssc", bufs=2, space="PSUM") as pssc,
                tc.tile_pool(name="psav", bufs=2, space="PSUM") as psav,
            ):
                for hf in range(2):
                    qs = hf * 1024
                    mask_sb = bigp.tile([128, 16, 1024], F32R, tag="big")
                    nc.sync.dma_start(
                        mask_sb[:],
                        maskT[:, ds(qs, 1024)].rearrange("(kb p) q -> p kb q", p=128),
                    )
                    for h in range(HPC):
                        off = 64 * (h % 2)
                        qt = qk_sb[off : off + 64, h // 2, :]
                        kt = qk_sb[off : off + 64, 2 + h // 2, :]
                        ps_av = psav.tile([65, 1024], mybir.dt.float32, tag="psav")
                        for kb in range(16):
                            ps_sc = pssc.tile([128, 1024], mybir.dt.float32, tag="pssc")
                            lhsT = kt[:, ts(kb, 128)]
                            for nn in range(2):
                                nc.tensor.matmul(
                                    ps_sc[:, ts(nn, 512)], lhsT,
                                    qt[:, ds(qs + nn * 512, 512)],
                                    start=True, stop=True,
                                )
                            at = attnp.tile([128, 1024], F32R, tag="attn")
                            nc.vector.tensor_tensor(
                                at[:], ps_sc[:], mask_sb[:, kb, :], mybir.AluOpType.add
                            )
                            nc.scalar.activation(at[:], at[:], AF.Exp)
                            for nn in range(2):
                                nc.tensor.matmul(
                                    ps_av[:, ts(nn, 512)], v_sb[:, kb, h, :],
                                    at[:, ts(nn, 512)],
                                    start=(kb == 0), stop=(kb == 15),
                                )
                        # normalize: vals = av[0:64] * (1 / av[64])
                        recip = smallp.tile([1, 1024], F32R, tag="recip")
                        with nc.allow_low_precision(
                            reason="float32r has fp32 bits; only PE matmul mode differs"
                        ):
                            nc.vector.reciprocal(recip[:], ps_av[64:65, :])
                        bc_sb = smallp.tile([64, 1024], F32R, tag="bc")
                        nc.gpsimd.partition_broadcast(bc_sb[:], recip[:])
                        nc.vector.tensor_tensor(
                            vals_sb[off : off + 64, h // 2, ds(qs, 1024)],
                            ps_av[0:64, :], bc_sb[:], mybir.AluOpType.mult,
                        )

            # ---- output projection (transposed, partial): outT = Wo^T vals^T ----
            with tc.tile_pool(name="psout", bufs=2, space="PSUM") as psout:
                for ob in range(8):
                    ps = psout.tile([128, S], mybir.dt.float32, tag="psout")
                    for kc in range(2):
                        lhsT = wo_sb[:, kc, ts(ob, 128)]
                        for nn in range(4):
                            nc.tensor.matmul(
                                ps[:, ts(nn, 512)], lhsT, vals_sb[:, kc, ts(nn, 512)],
                                start=(kc == 0), stop=(kc == 1),
                            )
                    oev = attnp.tile([128, S], mybir.dt.float32, tag="attn")
                    nc.scalar.activation(oev[:], ps[:], AF.Copy)
                    nc.sync.dma_start(outT[ds(ob * 128, 128), :], oev[:])

    nc.compile()
    return nc


def _prep_inputs(x, mask, W_qkv, b_qkv, W_o, b_o):
    """Host-side sharding/layout prep: slices, transposes, 1/sqrt(HD) folding."""
    scale = np.float32(1.0 / np.sqrt(HD))
    xT = [np.ascontiguousarray(x[b].T) for b in range(B)]
    maskT = np.ascontiguousarray(mask.T)
    in_maps = []
    for c in range(NCORES):
        b, g = divmod(c, HPC)
        heads = [HPC * g + i for i in range(HPC)]
        qcols = np.concatenate(
            [W_qkv[:, 192 * h : 192 * h + 64] for h in heads], axis=1) * scale
        kcols = np.concatenate(
            [W_qkv[:, 192 * h + 64 : 192 * h + 128] for h in heads], axis=1)
        wqk = np.ascontiguousarray(np.concatenate([qcols, kcols], axis=1))
        bq = np.concatenate([b_qkv[192 * h : 192 * h + 64] for h in heads]) * scale
        bk = np.concatenate([b_qkv[192 * h + 64 : 192 * h + 128] for h in heads])
        bqk_t = np.ascontiguousarray(
            np.concatenate([bq, bk]).reshape(4, 128).T)
        wv = np.ascontiguousarray(np.concatenate(
            [W_qkv[:, 192 * h + 128 : 192 * h + 192] for h in heads], axis=1))
        bv = np.ascontiguousarray(np.concatenate(
            [b_qkv[192 * h + 128 : 192 * h + 192] for h in heads])[None, :])
        wo = np.ascontiguousarray(W_o[256 * g : 256 * (g + 1), :])
        in_maps.append({
            "xT": xT[b], "wqk": wqk, "bqk": bqk_t, "wv": wv, "bv": bv,
            "wo": wo, "maskT": maskT,
        })
    return in_maps


def kernel(x, mask, W_qkv, b_qkv, W_o, b_o, _trace=False):
    x = np.asarray(x, dtype=np.float32)
    mask = np.asarray(mask, dtype=np.float32)
    W_qkv = np.asarray(W_qkv, dtype=np.float32)
    b_qkv = np.asarray(b_qkv, dtype=np.float32)
    W_o = np.asarray(W_o, dtype=np.float32)
    b_o = np.asarray(b_o, dtype=np.float32)

    if "nc" not in _CACHE:
        _CACHE["nc"] = _build()
    nc = _CACHE["nc"]

    in_maps = _prep_inputs(x, mask, W_qkv, b_qkv, W_o, b_o)
    res = run_bass_kernel_spmd(
        nc, in_maps, core_ids=list(range(NCORES)), trace=_trace
    )
    _CACHE["last_result"] = res

    out = np.empty((B, S, D), dtype=np.float32)
    for b in range(B):
        acc = res.results[HPC * b]["outT"].astype(np.float32)
        for g in range(1, HPC):
            acc = acc + res.results[HPC * b + g]["outT"]
        out[b] = acc.T + b_o
    return out
